# revision 39
# baseline (speedup 1.0000x reference)
"""GraphConv + BatchNorm + LeakyReLU fused layer on 8 Trainium2 NeuronCores.

Strategy (node/edge-partition sharding, v3):
  - Destination nodes are sharded across the 8 cores (6250 each). Within a
    core, dst nodes are assigned to 98 blocks of width 64 by balanced
    (LPT-style) packing on in-degree plus a swap-refinement pass, so every
    block's lo-half and hi-half edge counts fit uniform slot counts E and O
    (src rows are split at an adaptively chosen HALF so int16 gather indices
    work and (E, O) is minimal).
  - Each core fetches the bf16 source row for each edge with dma_gather
    (SWDGE, two queues: one per src half), one 256B descriptor per edge,
    into G tiles of [128 edge-lanes, slots, 128 feat]. All gather indices
    are staged in SBUF up-front (per-chunk index loads would queue behind
    the running gather transfer on the exclusive DMA engines).
  - The per-block segment sum is a PE matmul: aggT += G_t^T @ S_t where
    S_t[lane, d] = (dst_lane == d) is a [128, 64] one-hot built batched per
    chunk on DVE with a broadcast iota compare. Matmuls are emitted
    round-robin across blocks so consecutive matmuls hit different PSUM
    regions (hides the 173ns accumulation-chain latency).
  - Whole-chunk [128, chunk*64] stages: one aggT copy (ACT), one x1 pair of
    matmuls (lhsT reused), one leaky (ACT bias copy + DVE max), one x3
    matmul, one x3 store copy with per-chunk BN statistics accumulation.
  - Global BN stats go through a DRAM bounce + AllReduce; the final
    affine+leaky runs batched on [feat, node] and is stored as [128, npc_pad]
    f32 directly - the host transposes while unsharding.

kernel(**inputs) takes full-size numpy inputs, returns [50000, 128] float32.

Known dead-ends (measured in prior sessions; do not re-explore blindly):
  - fp8 gathers: 3.1% end-to-end rel err vs the 2e-2 gate (bf16 gives 0.47%).
  - indirect_dma_start (int32, single stream): corrupts multi-slot SBUF
    outputs in this stack; only the [P, 1]-index-per-partition form works.
  - Gather index tables must stay 8x-replicated across partition groups
    (the hardware descriptor generators read all 128 partitions; loading
    only group 0 crashes the device).
  - PSUM accumulation start=True zeroes the whole 2KB bank (zero-region):
    one start/stop per bank, block-consecutive groups otherwise.
  - E+O=16 slots (5.3% gain): arithmetically feasible at half=21904 with
    (E,O)=(7,9), but core 0 has only 168 edges of combined slack over 98
    blocks, so the per-block class sums must be packed near-exactly
    (windows of ~1-2 edges). The greedy swap refinement degrades to
    (8,10); an exact 2D bin-packing solver (ILP / min-cost flow over
    (lo_deg, hi_deg) pairs) in preprocess would unlock it with no
    device-side changes (slot counts are already parameterized).
"""
import sys

if "/opt/trn_rl_repo" not in sys.path:
    sys.path.insert(0, "/opt/trn_rl_repo")

import numpy as np
import ml_dtypes

import concourse.bass as bass
import concourse.mybir as mybir
import concourse.tile as tile
from concourse import bacc
from concourse import bass_utils
from concourse.masks import make_identity

F32 = mybir.dt.float32
BF16 = mybir.dt.bfloat16
I16 = mybir.dt.int16

N_NODES = 50000
N_CORES = 8
NPC = N_NODES // N_CORES          # 6250 nodes per core
BW = 64                           # dst-block width (S matrix width)
NBLK = (NPC + BW - 1) // BW       # 98 dst blocks per core
NPC_PAD = NBLK * BW               # 6272
LAST_BLK = NPC - BW * (NBLK - 1)  # 42 real nodes in the last block
ROWS_PAD = ((N_NODES + 127) // 128) * 128  # 50048
CHUNK = 6                         # dst blocks per gather pair
BN_EPS = 1e-5
NEG = 0.01


def _chunks(nblk, chunk):
    out = []
    b = 0
    while b < nblk:
        out.append((b, min(chunk, nblk - b)))
        b += out[-1][1]
    return out


def _wrap_idx(idx):
    """int16 gather index layout: position i -> [i % 16, i // 16] (one
    16-partition group; replicated to all 8 groups on-device)."""
    n = len(idx)
    assert n % 16 == 0
    return idx.reshape(n // 16, 16).T


def build_program(cfg):
    """Build the SPMD Bass program. cfg keys: n_cores, nblk, chunk, n_total,
    e_slots, o_slots, half, has_brel, has_blin."""
    ncores = cfg["n_cores"]
    nblk = cfg["nblk"]
    E = cfg["e_slots"]
    O = cfg["o_slots"]
    half = cfg["half"]
    chunk = cfg["chunk"]
    npc_pad = nblk * BW
    chunks = _chunks(nblk, chunk)
    nchunks = len(chunks)
    L = E + O
    nslots = nblk * L

    nc = bacc.Bacc("TRN2", target_bir_lowering=False, debug=False,
                   num_devices=ncores, num_swdge_queues=2)

    xb_d = nc.dram_tensor("xb", [ROWS_PAD, 128], BF16, kind="ExternalInput")
    xot_d = nc.dram_tensor("x_ownT", [128, npc_pad], BF16,
                           kind="ExternalInput")
    ixf_d = nc.dram_tensor("idx_f32", [16, nblk * L * 8], F32,
                           kind="ExternalInput")
    sel_d = nc.dram_tensor("sel16", [16, 128], F32, kind="ExternalInput")
    i0l_d = nc.dram_tensor("idx0_lo", [128, chunk * E * 8], I16,
                           kind="ExternalInput")
    i0h_d = nc.dram_tensor("idx0_hi", [128, chunk * O * 8], I16,
                           kind="ExternalInput")
    dv_d = nc.dram_tensor("dvals", [128, nslots], BF16, kind="ExternalInput")
    io_d = nc.dram_tensor("iota", [128, 128], BF16, kind="ExternalInput")
    wr_d = nc.dram_tensor("WrT", [128, 128], BF16, kind="ExternalInput")
    wo_d = nc.dram_tensor("WoT", [128, 128], BF16, kind="ExternalInput")
    wl_d = nc.dram_tensor("WlT", [128, 128], BF16, kind="ExternalInput")
    br_d = nc.dram_tensor("brel", [128, 1], F32, kind="ExternalInput")
    bl_d = nc.dram_tensor("blin", [128, 1], F32, kind="ExternalInput")
    ga_d = nc.dram_tensor("gamma", [128, 1], F32, kind="ExternalInput")
    be_d = nc.dram_tensor("beta", [128, 1], F32, kind="ExternalInput")
    out_d = nc.dram_tensor("out", [128, npc_pad], BF16, kind="ExternalOutput")

    inv_n = 1.0 / float(cfg["n_total"])

    with tile.TileContext(nc) as tc:
        with (
            tc.tile_pool(name="consts", bufs=1) as consts,
            tc.tile_pool(name="gp", bufs=3) as gp,
            tc.tile_pool(name="idxp", bufs=2) as idxp,
            tc.tile_pool(name="sp", bufs=3) as sp,
            tc.tile_pool(name="ps", bufs=2, space="PSUM") as ps,
            tc.tile_pool(name="misc", bufs=3) as misc,
            tc.tile_pool(name="x4p", bufs=4) as x4p,
            tc.tile_pool(name="big", bufs=1) as big,
            tc.tile_pool(name="dram", bufs=1, space="DRAM") as dram,
        ):
            # ---- constants / persistent tiles ----
            io_s = consts.tile([128, 128], BF16)
            wr_s = consts.tile([128, 128], BF16)
            wo_s = consts.tile([128, 128], BF16)
            wl_s = consts.tile([128, 128], BF16)
            br_s = consts.tile([128, 1], F32)
            bl_s = consts.tile([128, 1], F32)
            ga_s = consts.tile([128, 1], F32)
            be_s = consts.tile([128, 1], F32)
            dv_s = consts.tile([128, nslots], BF16)
            ix_s = consts.tile([128, nblk * L * 8], I16)
            sel_s = consts.tile([16, 128], F32)
            xot_s = big.tile([128, npc_pad], BF16)
            x3_s = big.tile([128, npc_pad], F32)
            sums = big.tile([128, nchunks + 1], F32)
            sqs = big.tile([128, nchunks + 1], F32)

            # gather indices arrive as an unreplicated [16, cols] f32
            # table; a PE matmul with a 16->128 selector replicates them
            # across the 8 partition groups (exact: values < 2^24), then a
            # DVE copy converts to the int16 layout dma_gather needs. This
            # replaces an 8x-replicated 9.5us DMA load with 2.4us.
            def issue_fetch(ci):
                b0, nb = chunks[ci]
                nlo = nb * E * 128
                nhi = nb * O * 128
                wlo = nb * E * 8
                whi = nb * O * 8
                if ci == 0:
                    # chunk 0's indices load pre-replicated: the PE path's
                    # cold-clock f32 matmuls would sit on the critical
                    # startup path
                    nc.sync.dma_start(ix_s[:, 0:wlo], i0l_d[:, 0:wlo])
                    nc.sync.dma_start(
                        ix_s[:, nblk * E * 8:nblk * E * 8 + whi],
                        i0h_d[:, 0:whi])
                else:
                    ixf_t = idxp.tile([16, chunk * L * 8], F32, tag="ixf")
                    nc.sync.dma_start(
                        ixf_t[:, 0:wlo + whi],
                        ixf_d[:, b0 * L * 8:b0 * L * 8 + wlo + whi])
                    rep_ps = ps.tile([128, 512], F32, tag="rep")
                    nc.tensor.matmul(rep_ps[:, 0:wlo], lhsT=sel_s[:],
                                     rhs=ixf_t[:, 0:wlo],
                                     start=True, stop=True)
                    nc.vector.tensor_copy(
                        ix_s[:, b0 * E * 8:(b0 + nb) * E * 8],
                        rep_ps[:, 0:wlo])
                    rep2_ps = ps.tile([128, 512], F32, tag="rep")
                    nc.tensor.matmul(rep2_ps[:, 0:whi], lhsT=sel_s[:],
                                     rhs=ixf_t[:, wlo:wlo + whi],
                                     start=True, stop=True)
                    nc.vector.tensor_copy(
                        ix_s[:, nblk * E * 8 + b0 * O * 8:
                             nblk * E * 8 + (b0 + nb) * O * 8],
                        rep2_ps[:, 0:whi])
                G_lo = gp.tile([128, chunk * E, 128], BF16, tag="Glo")
                G_hi = gp.tile([128, chunk * O, 128], BF16, tag="Ghi")
                nc.gpsimd.dma_gather(
                    out_ap=G_lo[:, 0:nb * E, :],
                    in_ap=xb_d[0:half, :],
                    idxs_ap=ix_s[:, b0 * E * 8:(b0 + nb) * E * 8],
                    num_idxs=nlo,
                    num_idxs_reg=nlo,
                    elem_size=128,
                    single_packet=False,
                    queue_num=0,
                )
                nc.gpsimd.dma_gather(
                    out_ap=G_hi[:, 0:nb * O, :],
                    in_ap=xb_d[half:ROWS_PAD, :],
                    idxs_ap=ix_s[:, nblk * E * 8 + b0 * O * 8:
                                 nblk * E * 8 + (b0 + nb) * O * 8],
                    num_idxs=nhi,
                    num_idxs_reg=nhi,
                    elem_size=128,
                    single_packet=False,
                    queue_num=1,
                )
                return G_lo, G_hi

            def issue_sbuild(ci):
                b0, nb = chunks[ci]
                ncols = nb * L
                s0 = b0 * L
                S = sp.tile([128, chunk * L, BW], BF16, tag="S")
                dvb = dv_s[:, s0:s0 + ncols]
                iota_bc = bass.AP(tensor=io_ap.tensor, offset=io_ap.offset,
                                  ap=[io_ap.ap[0], [0, ncols], io_ap.ap[1]])
                dv_bc = bass.AP(tensor=dvb.tensor, offset=dvb.offset,
                                ap=[dvb.ap[0], dvb.ap[1], [0, BW]])
                nc.vector.tensor_tensor(out=S[:, 0:ncols, :], in0=iota_bc,
                                        in1=dv_bc,
                                        op=mybir.AluOpType.is_equal)
                return S

            G_pend = issue_fetch(0)

            nc.sync.dma_start(sel_s[:], sel_d[:])
            nc.sync.dma_start(dv_s[:], dv_d[:])
            nc.scalar.dma_start(io_s[:], io_d[:])
            nc.scalar.dma_start(wr_s[:], wr_d[:])
            nc.scalar.dma_start(wo_s[:], wo_d[:])
            nc.scalar.dma_start(wl_s[:], wl_d[:])
            nc.scalar.dma_start(br_s[:], br_d[:])
            nc.scalar.dma_start(bl_s[:], bl_d[:])
            nc.scalar.dma_start(ga_s[:], ga_d[:])
            nc.scalar.dma_start(be_s[:], be_d[:])
            nc.scalar.dma_start(xot_s[:], xot_d[:])

            # warm the ACT function table with the sqrt_and_friends set
            # (it also contains identity/square/relu/copy), so the BN tail's
            # Sqrt does not trigger a 1.3us table reload on the critical
            # stats chain. scale=0 means the input is never read.
            warm = consts.tile([128, 1], F32)
            nc.scalar.activation(warm[:], br_s[:],
                                 mybir.ActivationFunctionType.Sqrt,
                                 bias=1.0, scale=0.0)

            io_ap = io_s[:, 0:BW]
            S_pend = issue_sbuild(0)

            for ci, (b0, nb) in enumerate(chunks):
                nd = nb * BW            # dst columns in this chunk
                d0 = b0 * BW
                # real (non-pad) dst columns in this chunk
                ndr = nd - (BW - LAST_BLK) if b0 + nb == nblk else nd
                G_lo, G_hi = G_pend
                S = S_pend
                if ci + 1 < nchunks:
                    G_pend = issue_fetch(ci + 1)
                    S_pend = issue_sbuild(ci + 1)

                # ---- segment-sum matmuls into one [128, nd] PSUM tile ----
                # slot-outer / block-inner: consecutive matmuls hit different
                # PSUM regions (hides the accumulation-chain latency).
                # S slot order per chunk: per block, E lo slots then O hi.
                # t-outer / b-inner so consecutive matmuls hit different
                # PSUM regions (hides the 173ns accumulation drain). PSUM
                # zeroing is zero-region (bank) granular: exactly one
                # start (first matmul) marks the whole bank pending-zero
                # and one stop (last matmul) closes the group.
                agg_ps = ps.tile([128, chunk * BW], F32, tag="agg")
                for t in range(L):
                    for b in range(nb):
                        if t < E:
                            lhsT = G_lo[:, b * E + t, :]
                        else:
                            lhsT = G_hi[:, b * O + (t - E), :]
                        nc.tensor.matmul(
                            agg_ps[:, b * BW:(b + 1) * BW],
                            lhsT=lhsT,
                            rhs=S[:, b * L + t, :],
                            start=(t == 0 and b == 0),
                            stop=(t == L - 1 and b == nb - 1),
                            skip_group_check=True)
                aggT = misc.tile([128, chunk * BW], BF16, tag="aggT")
                nc.scalar.copy(aggT[:, 0:nd], agg_ps[:, 0:nd])

                # ---- x1^T = W_rel^T.T @ aggT + W_root^T.T @ x_own^T ----
                x1_ps = ps.tile([128, chunk * BW], F32, tag="x1")
                nc.tensor.matmul(x1_ps[:, 0:nd], lhsT=wr_s[:],
                                 rhs=aggT[:, 0:nd], start=True, stop=False)
                nc.tensor.matmul(x1_ps[:, 0:nd], lhsT=wo_s[:],
                                 rhs=xot_s[:, d0:d0 + nd],
                                 start=False, stop=True)

                # x2 = leaky(x1 + b_rel): ACT copy (bias) + DVE one-op leaky
                x2_sb = misc.tile([128, chunk * BW], BF16, tag="x2")
                v_sb = misc.tile([128, chunk * BW], BF16, tag="v")
                brel = br_s[:] if cfg["has_brel"] else 0.0
                nc.scalar.activation(
                    v_sb[:, 0:nd], x1_ps[:, 0:nd],
                    mybir.ActivationFunctionType.Identity,
                    bias=brel, scale=1.0)
                nc.vector.scalar_tensor_tensor(
                    out=x2_sb[:, 0:nd], in0=v_sb[:, 0:nd], scalar=NEG,
                    in1=v_sb[:, 0:nd],
                    op0=mybir.AluOpType.mult, op1=mybir.AluOpType.max)

                # x3^T = W_lin^T.T @ x2
                x3_ps = ps.tile([128, chunk * BW], F32, tag="x3")
                nc.tensor.matmul(x3_ps[:, 0:nd], lhsT=wl_s[:],
                                 rhs=x2_sb[:, 0:nd], start=True, stop=True)

                # copy to x3_s (+ b_lin) accumulating per-feature sums over
                # the real columns only
                blin = bl_s[:] if cfg["has_blin"] else 0.0
                nc.scalar.activation(
                    x3_s[:, d0:d0 + ndr], x3_ps[:, 0:ndr],
                    mybir.ActivationFunctionType.Identity,
                    bias=blin, scale=1.0,
                    accum_out=sums[:, ci:ci + 1])
                junk = misc.tile([128, chunk * BW], BF16, tag="junk")
                nc.scalar.activation(
                    junk[:, 0:ndr], x3_s[:, d0:d0 + ndr],
                    mybir.ActivationFunctionType.Square,
                    accum_out=sqs[:, ci:ci + 1])

            if NPC < npc_pad:
                nc.vector.memset(x3_s[:, NPC:npc_pad], 0.0)

            # ---- global BN statistics via AllReduce ----
            stat2 = consts.tile([128, 2], F32)
            nc.vector.tensor_reduce(stat2[:, 0:1], sums[:, 0:nchunks],
                                    axis=mybir.AxisListType.X,
                                    op=mybir.AluOpType.add)
            nc.vector.tensor_reduce(stat2[:, 1:2], sqs[:, 0:nchunks],
                                    axis=mybir.AxisListType.X,
                                    op=mybir.AluOpType.add)
            cc_in = dram.tile([128, 2], F32)
            cc_out = dram.tile([128, 2], F32)
            nc.sync.dma_start(cc_in[:], stat2[:])
            if ncores > 1 and not cfg.get("no_cc"):
                nc.gpsimd.collective_compute(
                    "AllReduce",
                    mybir.AluOpType.add,
                    replica_groups=[list(range(ncores))],
                    ins=[cc_in[:].opt()],
                    outs=[cc_out[:].opt()],
                )
                red = cc_out
            else:
                red = cc_in
            stat_r = consts.tile([128, 2], F32)
            nc.sync.dma_start(stat_r[:], red[:])

            mean = consts.tile([128, 1], F32)
            ex2 = consts.tile([128, 1], F32)
            var = consts.tile([128, 1], F32)
            rstd = consts.tile([128, 1], F32)
            scl = consts.tile([128, 1], F32)
            bia = consts.tile([128, 1], F32)
            tmp1 = consts.tile([128, 1], F32)
            epsv = consts.tile([128, 1], F32)
            nc.vector.memset(epsv[:], BN_EPS)
            nc.vector.tensor_scalar_mul(mean[:], stat_r[:, 0:1], inv_n)
            nc.vector.tensor_tensor(out=tmp1[:], in0=mean[:], in1=mean[:],
                                    op=mybir.AluOpType.mult)
            nc.vector.scalar_tensor_tensor(
                out=var[:], in0=stat_r[:, 1:2], scalar=inv_n, in1=tmp1[:],
                op0=mybir.AluOpType.mult, op1=mybir.AluOpType.subtract)
            nc.scalar.activation(rstd[:], var[:],
                                 mybir.ActivationFunctionType.Sqrt,
                                 bias=epsv[:], scale=1.0)
            nc.vector.reciprocal(rstd[:], rstd[:])
            nc.vector.tensor_tensor(out=scl[:], in0=ga_s[:], in1=rstd[:],
                                    op=mybir.AluOpType.mult)
            nc.vector.tensor_tensor(out=tmp1[:], in0=mean[:], in1=scl[:],
                                    op=mybir.AluOpType.mult)
            nc.vector.tensor_sub(bia[:], be_s[:], tmp1[:])

            # ---- normalize + leaky (batched), store [feat, node] bf16 ----
            # (the host transposes + upcasts the per-core [128, npc_pad]
            # output while unsharding; no on-device transposes needed).
            # Graded slice sizes: a small first slice lets the first store
            # start early; leaky writes bf16 staging tiles to halve the
            # store bytes.
            slices = [256, 512]
            while sum(slices) < npc_pad:
                slices.append(min(1024, npc_pad - sum(slices)))
            done = 0
            for w in slices:
                lo = done
                hi = done + w
                nc.scalar.activation(x3_s[:, lo:hi], x3_s[:, lo:hi],
                                     mybir.ActivationFunctionType.Identity,
                                     bias=bia[:], scale=scl[:])
                x4_q = x4p.tile([128, 1024], BF16, tag="x4")
                nc.vector.scalar_tensor_tensor(
                    out=x4_q[:, 0:w], in0=x3_s[:, lo:hi], scalar=NEG,
                    in1=x3_s[:, lo:hi],
                    op0=mybir.AluOpType.mult, op1=mybir.AluOpType.max)
                nc.sync.dma_start(out_d[:, lo:hi], x4_q[:, 0:w])
                done += w

    nc.compile()
    return nc


def preprocess(x, edge_index, cfg):
    """Host-side sharding: balanced dst blocks + per-core edge/index arrays.

    Returns (per_core_inputs, perm). Sets cfg['half'], cfg['e_slots'],
    cfg['o_slots'].
    """
    ncores = cfg["n_cores"]
    nblk = cfg["nblk"]
    n = x.shape[0]
    npc = cfg["npc"]
    npc_pad = nblk * BW

    src = np.asarray(edge_index[0], dtype=np.int64)
    dst = np.asarray(edge_index[1], dtype=np.int64)
    core = dst // npc
    loc = dst - core * npc

    xb = np.zeros((ROWS_PAD, 128), dtype=ml_dtypes.bfloat16)
    xb[:n] = x.astype(ml_dtypes.bfloat16)

    # choose the lo/hi split point: lo rows and hi rows must both be
    # int16-addressable; aim lo ~45% so lo fits E=8 slots per block with
    # slack and hi fits O=9
    half = 23040
    assert half < 32768 and ROWS_PAD - half < 32768
    cfg["half"] = half
    E_t, O_t = 8, 9

    lo_e = (src < half).astype(np.int64)
    deg = np.zeros((ncores, npc), dtype=np.int64)
    dlo = np.zeros((ncores, npc), dtype=np.int64)
    np.add.at(deg, (core, loc), 1)
    np.add.at(dlo, (core, loc), lo_e)
    dhi = deg - dlo

    blk_of = np.empty((ncores, npc), dtype=np.int64)
    pos_of = np.empty((ncores, npc), dtype=np.int64)
    caps = np.full(nblk, BW, dtype=np.int64)
    caps[nblk - 1] = LAST_BLK
    e_need = 1
    o_need = 1
    for c in range(ncores):
        order = np.argsort(-deg[c], kind="stable")
        sums_lo = np.zeros(nblk, dtype=np.int64)
        sums_hi = np.zeros(nblk, dtype=np.int64)
        fill = np.zeros(nblk, dtype=np.int64)
        ptr = 0
        while ptr < npc:
            open_b = np.where(fill < caps)[0]
            k = min(len(open_b), npc - ptr)
            ob = open_b[np.argsort(sums_lo[open_b] + sums_hi[open_b],
                                   kind="stable")[:k]]
            nodes = order[ptr:ptr + k]
            blk_of[c, nodes] = ob
            pos_of[c, nodes] = fill[ob]
            sums_lo[ob] += dlo[c, nodes]
            sums_hi[ob] += dhi[c, nodes]
            fill[ob] += 1
            ptr += k
        # swap refinement for each class independently: bring every block's
        # lo count <= 128*E_t and hi count <= 128*O_t
        nodes_by_blk = [list(np.where(blk_of[c] == b)[0])
                        for b in range(nblk)]
        def do_swap(h, l, ai, bi):
            a = nodes_by_blk[h][ai]
            bnd = nodes_by_blk[l][bi]
            blk_of[c, a], blk_of[c, bnd] = l, h
            pos_of[c, a], pos_of[c, bnd] = pos_of[c, bnd], pos_of[c, a]
            nodes_by_blk[h][ai] = bnd
            nodes_by_blk[l][bi] = a
            ldiff = int(dlo[c, a] - dlo[c, bnd])
            hdiff = int(dhi[c, a] - dhi[c, bnd])
            sums_lo[h] -= ldiff
            sums_lo[l] += ldiff
            sums_hi[h] -= hdiff
            sums_hi[l] += hdiff

        # two refinement passes per class: partial-progress swaps (node
        # degrees differ by far less than the needed reduction, so one swap
        # rarely closes the gap alone) with a guard keeping the other class
        # at or under its own target
        for cls_sums, cls_deg, target, osums, odeg, otarget in (
                (sums_lo, dlo[c], 128 * E_t, sums_hi, dhi[c], 128 * O_t),
                (sums_hi, dhi[c], 128 * O_t, sums_lo, dlo[c], 128 * E_t)):
            if cls_deg.sum() > target * nblk:
                continue  # infeasible; slot count will grow below
            for _ in range(5000):
                h = int(np.argmax(cls_sums))
                r = cls_sums[h] - target
                if r <= 0:
                    break
                done_swap = False
                for l in np.argsort(cls_sums)[:16]:
                    if l == h:
                        continue
                    dh_ = cls_deg[nodes_by_blk[h]]
                    dl_ = cls_deg[nodes_by_blk[l]]
                    oh_ = odeg[nodes_by_blk[h]]
                    ol_ = odeg[nodes_by_blk[l]]
                    for ai in np.argsort(-dh_):
                        da = dh_[ai]
                        ok = np.where(
                            (dl_ < da) &
                            (cls_sums[l] + da - dl_ <= target) &
                            (osums[l] + oh_[ai] - ol_ <= otarget) &
                            (osums[h] - oh_[ai] + ol_ <=
                             max(otarget, osums[h])))[0]
                        if len(ok):
                            # prefer the exact reduction if available,
                            # otherwise take the biggest step
                            good = ok[dl_[ok] >= da - r]
                            bi = int(good[np.argmax(dl_[good])]) if len(good) \
                                else int(ok[np.argmin(dl_[ok])])
                            do_swap(h, l, ai, bi)
                            done_swap = True
                            break
                    if done_swap:
                        break
                if not done_swap:
                    break
        e_need = max(e_need, int(np.ceil(sums_lo.max() / 128)))
        o_need = max(o_need, int(np.ceil(sums_hi.max() / 128)))
    E = max(e_need, 1)
    O = max(o_need, 1)
    cfg["e_slots"] = E
    cfg["o_slots"] = O
    L = E + O
    nslots = nblk * L

    # per-edge placement: within (core, block), lo edges fill the E lo
    # slots, hi edges the O hi slots
    e_blk = blk_of[core, loc]
    e_dloc = pos_of[core, loc]
    key = (core * nblk + e_blk) * 2 + (1 - lo_e)
    order_e = np.argsort(key, kind="stable")
    key_s = key[order_e]
    src_s = src[order_e]
    dloc_s = e_dloc[order_e]
    ngroups = ncores * nblk * 2
    counts = np.bincount(key_s, minlength=ngroups)
    starts = np.zeros(ngroups + 1, dtype=np.int64)
    np.cumsum(counts, out=starts[1:])
    j = np.arange(len(src_s)) - starts[key_s]   # rank within (core, blk, half)
    c_e = key_s // (2 * nblk)
    b_e = (key_s // 2) % nblk
    h_e = key_s % 2

    ilo = np.zeros((ncores, nblk, E * 128), dtype=np.int64)
    ihi = np.zeros((ncores, nblk, O * 128), dtype=np.int64)
    dv_all = np.full((ncores, nblk, L, 128), 255, dtype=np.int64)
    mlo = h_e == 0
    ilo[c_e[mlo], b_e[mlo], j[mlo]] = src_s[mlo]
    ihi[c_e[~mlo], b_e[~mlo], j[~mlo]] = src_s[~mlo] - half
    dv_all[c_e[mlo], b_e[mlo], j[mlo] // 128, j[mlo] % 128] = dloc_s[mlo]
    dv_all[c_e[~mlo], b_e[~mlo], E + j[~mlo] // 128, j[~mlo] % 128] = \
        dloc_s[~mlo]

    per_core = []
    perm = np.full((ncores, npc_pad), -1, dtype=np.int64)
    for c in range(ncores):
        il_parts = []
        ih_parts = []
        for b in range(nblk):
            il_parts.append(_wrap_idx(ilo[c, b]))
            ih_parts.append(_wrap_idx(ihi[c, b]))
        # chunk-ordered: [chunk0 lo | chunk0 hi | chunk1 lo | ...]
        parts = []
        for (b, nb) in _chunks(nblk, CHUNK):
            parts.extend(il_parts[b:b + nb])
            parts.extend(ih_parts[b:b + nb])
        idx_f32 = np.concatenate(parts, axis=1).astype(np.float32)
        nb0 = _chunks(nblk, CHUNK)[0][1]
        idx0_lo = np.tile(np.concatenate(il_parts[0:nb0], axis=1),
                          (8, 1)).astype(np.int16)
        idx0_hi = np.tile(np.concatenate(ih_parts[0:nb0], axis=1),
                          (8, 1)).astype(np.int16)
        # dvals: [128 lanes, nblk*L slots]
        dv = np.ascontiguousarray(
            dv_all[c].reshape(nblk * L, 128).T)
        nodes = np.arange(npc, dtype=np.int64)
        slot = blk_of[c] * BW + pos_of[c]
        perm[c, slot] = nodes + c * npc
        xoT = np.zeros((128, npc_pad), dtype=ml_dtypes.bfloat16)
        xoT[:, slot] = xb[nodes + c * npc].T
        per_core.append({
            "xb": xb,
            "x_ownT": xoT,
            "idx_f32": idx_f32,
            "idx0_lo": idx0_lo,
            "idx0_hi": idx0_hi,
            "dvals": dv.astype(ml_dtypes.bfloat16),
        })
    return per_core, perm


_PROGRAM_CACHE = {}


def run(x, edge_index, W_rel, b_rel, W_root, W_lin, b_lin, gamma, beta, cfg):
    per_core, perm = preprocess(x, edge_index, cfg)
    cfg["has_brel"] = bool(np.any(b_rel != 0))
    cfg["has_blin"] = bool(np.any(b_lin != 0))

    iota = np.tile(np.arange(128, dtype=np.float32), (128, 1))
    sel = (np.arange(128)[None, :] % 16 ==
           np.arange(16)[:, None]).astype(np.float32)
    shared = {
        "iota": iota.astype(ml_dtypes.bfloat16),
        "sel16": sel,
        "WrT": np.ascontiguousarray(W_rel.T).astype(ml_dtypes.bfloat16),
        "WoT": np.ascontiguousarray(W_root.T).astype(ml_dtypes.bfloat16),
        "WlT": np.ascontiguousarray(W_lin.T).astype(ml_dtypes.bfloat16),
        "brel": b_rel.reshape(128, 1).astype(np.float32),
        "blin": b_lin.reshape(128, 1).astype(np.float32),
        "gamma": gamma.reshape(128, 1).astype(np.float32),
        "beta": beta.reshape(128, 1).astype(np.float32),
    }
    in_maps = [dict(m, **shared) for m in per_core]

    key = (cfg["n_cores"], cfg["nblk"], cfg["e_slots"], cfg["o_slots"],
           cfg["half"], cfg["chunk"], cfg["has_brel"], cfg["has_blin"])
    if key not in _PROGRAM_CACHE:
        _PROGRAM_CACHE[key] = build_program(cfg)
    nc = _PROGRAM_CACHE[key]

    res = bass_utils.run_bass_kernel_spmd(
        nc, in_maps, core_ids=list(range(cfg["n_cores"])))
    n = x.shape[0]
    out = np.empty((n, 128), dtype=np.float32)
    for c in range(cfg["n_cores"]):
        o = np.asarray(res.results[c]["out"]).astype(np.float32).T
        m = perm[c] >= 0
        out[perm[c][m]] = o[m]
    return out


def kernel(x, edge_index, batch, W_rel, b_rel, W_root, W_lin, b_lin, gamma,
           beta):
    x = np.asarray(x, dtype=np.float32)
    cfg = {
        "n_cores": N_CORES,
        "npc": NPC,
        "nblk": NBLK,
        "chunk": CHUNK,
        "n_total": N_NODES,
    }
    return run(x, np.asarray(edge_index), np.asarray(W_rel, dtype=np.float32),
               np.asarray(b_rel, dtype=np.float32),
               np.asarray(W_root, dtype=np.float32),
               np.asarray(W_lin, dtype=np.float32),
               np.asarray(b_lin, dtype=np.float32),
               np.asarray(gamma, dtype=np.float32),
               np.asarray(beta, dtype=np.float32), cfg)
